# revision 13
# baseline (speedup 1.0000x reference)
"""ColBERT maxsim scoring kernel for Trainium2 (8 NeuronCores, SPMD).

Problem: Q [128, 32, 128] f32, D [1024, 220, 128] f32, D_mask [1024, 220] i32,
nway=8.  out[b] = sum_q max_k where(mask[b,k], D[b] @ Q[b//8].T, -9999)[k, q]
for b in 0..1024.

Sharding: data-parallel over docs. Core c handles docs [128c, 128c+128) and
the matching 16 query batches.

v2 design notes (from the v1 trace):
  - DMA engines are the hard floor: 16 engines x ~22.5 B/ns => ~34.6us to
    stream the 14.4MB of f32 D rows per core (cast to bf16 in flight).
  - v1 lost ~10us of startup (first D packet at 10.5us) and ~22us of tail
    (a full 32-doc megachunk of transposes/copies/scores after the last DMA).
  - v2 uses variable-size chunks [8, 32, 32, 32, 16, 8] docs: a small first
    chunk so the PE starts early, a small last chunk so the tail is short.
    Chunk0 rides the HWDGE (sync) queue, the rest SWDGE (gpsimd); each chunk
    is split into sub-DMAs for fine-grained unlock.
  - Blocked DMA layout per chunk: partition p holds a RUN contiguous rows
    (RUN=55 at 128 partitions for 32-doc chunks; RUN=16/32 at 110 partitions
    for 8/16-doc chunks) so each descriptor reads RUN*512B contiguous.
  - PE transposes tiles to a NATURAL-order D^T tile (dt col r = chunk row r)
    via strided bank copies, so score matmuls read contiguous 440-col slabs.
  - Per 8-doc group: K=5 bias matmul (selector x [ones; mask]) initializes
    the PSUM bank with -9999*(1-mask), then 4 col-tiled matmuls add scores
    for the 4 doc pairs. reduce_max per doc -> mx [128, 32]; final bsel
    matmul sums 32-query partition blocks -> out [4, 32].
  - Scores of chunk c are interleaved between transpose banks of chunk c+1
    to keep the PE dense (p-state ramp) and hide LDWEIGHTS.
"""

import numpy as np

import concourse.bacc as bacc
import concourse.mybir as mybir
from concourse import bass_utils
from concourse.tile import TileContext

F32 = mybir.dt.float32
BF16 = mybir.dt.bfloat16

N_CORES = 8
B = 128          # query batches
QLEN = 32
DIM = 128
NWAY = 8
DLEN = 220
DOCS_PER_CORE = (B * NWAY) // N_CORES          # 128
ROWS_PER_CORE = DOCS_PER_CORE * DLEN           # 28160
GROUPS_PER_CORE = DOCS_PER_CORE // NWAY        # 16
BIG = 9999.0

# (docs, partitions, run, sub-splits in w units); 440 %% run == 0 so that
# doc pairs are partition-aligned in the blocked layout (run rows/partition)
CHUNKS = [
    (8,  88,  20, (10, 10)),
    (32, 128, 55, (55,)),
    (32, 128, 55, (55,)),
    (32, 128, 55, (55,)),
    (16, 88,  40, (40,)),
    (8,  88,  20, (10, 10)),
]
assert sum(c[0] for c in CHUNKS) == DOCS_PER_CORE

_CACHE = {}


def _build_module():
    """Trace + compile the per-core bass module (same program on all cores)."""
    if "nc" in _CACHE:
        return _CACHE["nc"]

    nc = bacc.Bacc("TRN2", target_bir_lowering=False, debug=False)

    d_dram = nc.dram_tensor("d_in", [ROWS_PER_CORE, DIM], F32, kind="ExternalInput")
    q_dram = nc.dram_tensor("q_in", [GROUPS_PER_CORE * QLEN, DIM], BF16,
                            kind="ExternalInput")
    m_dram = nc.dram_tensor("m_in", [DOCS_PER_CORE, DLEN], BF16,
                            kind="ExternalInput")
    sel_dram = nc.dram_tensor("sel5", [5, 128], BF16, kind="ExternalInput")
    id_dram = nc.dram_tensor("ident", [128, 128], BF16, kind="ExternalInput")
    ones_dram = nc.dram_tensor("ones_row", [1, 32 * DLEN], BF16,
                               kind="ExternalInput")
    bsel_dram = nc.dram_tensor("bsel", [128, 4], F32, kind="ExternalInput")
    out_dram = nc.dram_tensor("outp", [4, 32], F32, kind="ExternalOutput")

    with TileContext(nc) as tc:
        with (
            tc.tile_pool(name="const", bufs=1) as cpool,
            tc.tile_pool(name="draw", bufs=4) as draw_pool,
            tc.tile_pool(name="dt", bufs=2) as dt_pool,
            tc.tile_pool(name="trps", bufs=3, space="PSUM") as trps_pool,
            tc.tile_pool(name="score", bufs=3, space="PSUM") as score_pool,
            tc.tile_pool(name="fin", bufs=1, space="PSUM") as fin_pool,
        ):
            # --- chunk0 D load first in the SWDGE (gpsimd) queue
            row0 = 0
            chunk_draws = []
            docs0, P0, RUN0, SPLIT0 = CHUNKS[0]
            rows0 = docs0 * DLEN
            d_v0 = d_dram.ap()[0:rows0, :].rearrange("(p w) d -> p w d", p=P0)
            draw0 = draw_pool.tile([P0, RUN0 * 128], BF16)
            w0 = 0
            for wlen in SPLIT0:
                nc.gpsimd.dma_start(
                    out=draw0[:, 128 * w0:128 * (w0 + wlen)],
                    in_=d_v0[:, w0:w0 + wlen, :])
                w0 += wlen
            chunk_draws.append(draw0)
            row0 = rows0

            ident = cpool.tile([128, 128], BF16)
            nc.sync.dma_start(out=ident[:, :], in_=id_dram.ap())
            qraw = cpool.tile([128, 4 * 128], BF16)
            nc.sync.dma_start(
                out=qraw[:, :],
                in_=q_dram.ap().rearrange("(n p) d -> p n d", p=128),
            )
            sel5 = cpool.tile([5, 128], BF16)
            nc.sync.dma_start(out=sel5[:, :], in_=sel_dram.ap())
            bsel = cpool.tile([128, 4], F32)
            nc.sync.dma_start(out=bsel[:, :], in_=bsel_dram.ap())

            # maskf rows: 0 = ones; 1+j = mask of pair j, cols (g, t, k)
            maskf = cpool.tile([5, GROUPS_PER_CORE * 2 * DLEN], BF16)
            nc.sync.dma_start(out=maskf[0:1, :], in_=ones_dram.ap())
            nc.sync.dma_start(
                out=maskf[1:5, :],
                in_=m_dram.ap().rearrange("(g j t) k -> j g t k", g=16, t=2),
            )

            # --- remaining chunks on the SWDGE (gpsimd) queue
            for (docs, P, RUN, SPLIT) in CHUNKS[1:]:
                rows = docs * DLEN
                d_v = d_dram.ap()[row0:row0 + rows, :].rearrange(
                    "(p w) d -> p w d", p=P)
                draw = draw_pool.tile([P, RUN * 128], BF16)
                w0 = 0
                for wlen in SPLIT:
                    nc.gpsimd.dma_start(
                        out=draw[:, 128 * w0:128 * (w0 + wlen)],
                        in_=d_v[:, w0:w0 + wlen, :])
                    w0 += wlen
                chunk_draws.append(draw)
                row0 += rows

            # --- Q^T: 4 natural [128, 128] chunks -> PE transpose -> qt
            qpsum = fin_pool.tile([128, 512], BF16)
            for i in range(4):
                nc.tensor.transpose(
                    qpsum[:, 128 * i:128 * (i + 1)],
                    qraw[:, 128 * i:128 * (i + 1)],
                    ident[:, :],
                )
            qt = cpool.tile([128, GROUPS_PER_CORE * QLEN], BF16)
            nc.vector.tensor_copy(qt[:, :], qpsum[:, :])

            mx = cpool.tile([128, 32], F32)

            def make_group_emitter(G, dtv, gl, P, RUN):
                """Score group: global group G, local group gl in chunk.

                dtv = dt viewed [128, P, RUN]; pair j occupies partitions
                [ (1760*gl + 440*j)//RUN , +440//RUN ) of the blocked layout,
                streamed (p outer, w inner) = natural token order."""
                npair = 440 // RUN
                def emit():
                    ps = score_pool.tile([128, 2 * DLEN], F32)
                    nc.tensor.matmul(
                        ps[:, :],
                        lhsT=sel5[:, :],
                        rhs=maskf[:, 2 * DLEN * G:2 * DLEN * (G + 1)],
                        start=True, stop=False,
                    )
                    for j in range(4):
                        p0 = (8 * DLEN * gl + 2 * DLEN * j) // RUN
                        nc.tensor.matmul(
                            ps[32 * j:32 * (j + 1), :],
                            lhsT=qt[:, QLEN * G:QLEN * (G + 1)],
                            rhs=dtv[:, p0:p0 + npair, :],
                            start=False, stop=(j == 3),
                            tile_position=(0, 32 * j),
                            skip_group_check=True,
                        )
                    for t in range(2):
                        s = G * 2 + t
                        nc.vector.tensor_reduce(
                            mx[:, s:s + 1],
                            ps[:, DLEN * t:DLEN * (t + 1)],
                            axis=mybir.AxisListType.X,
                            op=mybir.AluOpType.max,
                        )
                return emit

            # --- banks + scores, interleaved across chunks
            copy_flip = [0]
            G0 = 0           # global group index at chunk start

            def emit_bank(chunk_idx, j, P, T, draw, dt):
                ntr = min(4, T - 4 * j)
                ptr = trps_pool.tile([128, 4 * P], BF16)
                for i in range(ntr):
                    w = 4 * j + i
                    nc.tensor.transpose(
                        ptr[:, P * i:P * (i + 1)],
                        draw[:, 128 * w:128 * (w + 1)],
                        ident[0:P, 0:P],
                    )
                # contiguous copy; DT col = P*w + p
                if copy_flip[0] % 2 == 1:
                    nc.scalar.copy(dt[:, 4 * P * j:4 * P * j + ntr * P],
                                   ptr[:, 0:ntr * P])
                else:
                    nc.vector.tensor_copy(dt[:, 4 * P * j:4 * P * j + ntr * P],
                                          ptr[:, 0:ntr * P])
                copy_flip[0] += 1

            for ci, (docs, P, RUN, SPLIT) in enumerate(CHUNKS):
                T = RUN                       # tiles per chunk == run length
                draw = chunk_draws[ci]
                dt = dt_pool.tile([128, docs * DLEN], BF16)
                nbanks = (T + 3) // 4
                for j in range(nbanks):
                    emit_bank(ci, j, P, T, draw, dt)
                dtv = dt[:, :].rearrange("d (w p) -> d p w", p=P)
                for gl in range(docs // 8):
                    make_group_emitter(G0 + gl, dtv, gl, P, RUN)()
                G0 += docs // 8

            fpsum = fin_pool.tile([4, 32], F32)
            nc.tensor.matmul(fpsum[:, :], lhsT=bsel[:, :], rhs=mx[:, :],
                             start=True, stop=True)
            fout = cpool.tile([4, 32], F32)
            nc.scalar.copy(fout[:, :], fpsum[:, :])
            nc.sync.dma_start(out=out_dram.ap(), in_=fout[:, :])

    nc.compile()
    _CACHE["nc"] = nc
    return nc


def _host_constants():
    j = np.arange(4)
    m = np.arange(128)
    import ml_dtypes
    sel5 = np.zeros((5, 128), np.float32)
    sel5[0] = -BIG
    sel5[1:5] = BIG * (m[None, :] // 32 == j[:, None])
    sel5 = sel5.astype(ml_dtypes.bfloat16)
    bsel = (m[:, None] // 32 == j[None, :]).astype(np.float32)
    ident = np.eye(128, dtype=ml_dtypes.bfloat16)
    ones_row = np.ones((1, 32 * DLEN), dtype=ml_dtypes.bfloat16)
    return sel5, bsel, ident, ones_row


def kernel(Q, D, D_mask, nway):
    assert int(nway) == NWAY
    Q = np.ascontiguousarray(np.asarray(Q, dtype=np.float32))
    D = np.ascontiguousarray(np.asarray(D, dtype=np.float32))
    D_mask = np.ascontiguousarray(np.asarray(D_mask, dtype=np.int32))

    nc = _build_module()
    sel5, bsel, ident, ones_row = _host_constants()

    in_maps = []
    for c in range(N_CORES):
        dc = D[c * DOCS_PER_CORE:(c + 1) * DOCS_PER_CORE].reshape(
            ROWS_PER_CORE, DIM)
        import ml_dtypes
        qc = Q[c * GROUPS_PER_CORE:(c + 1) * GROUPS_PER_CORE].reshape(
            GROUPS_PER_CORE * QLEN, DIM).astype(ml_dtypes.bfloat16)
        m_c = D_mask[c * DOCS_PER_CORE:(c + 1) * DOCS_PER_CORE].astype(
            ml_dtypes.bfloat16)
        in_maps.append({
            "d_in": dc, "q_in": qc, "m_in": m_c,
            "sel5": sel5, "bsel": bsel, "ident": ident, "ones_row": ones_row,
        })

    res = bass_utils.run_bass_kernel_spmd(nc, in_maps,
                                          core_ids=list(range(N_CORES)))

    # out[j, s] = doc (8*(s//2) + 2*j + s%2) within the core
    s = np.arange(32)
    j = np.arange(4)
    doc_idx = 8 * (s[None, :] // 2) + 2 * j[:, None] + (s[None, :] % 2)
    out = np.empty(B * NWAY, np.float32)
    for c in range(N_CORES):
        per_core = np.empty(DOCS_PER_CORE, np.float32)
        per_core[doc_idx.ravel()] = res.results[c]["outp"].ravel()
        out[c * DOCS_PER_CORE:(c + 1) * DOCS_PER_CORE] = per_core
    return out
